# revision 9
# baseline (speedup 1.0000x reference)
"""Masked attention on 8 TRN2 NeuronCores — pure data-parallel over batch.

Full inputs:  q,k,v (16,2048,128) f32, mask (16,2048,2048) bool.
Output:       (16,2048,128) f32.

Per core (2 batches): computes transposed scores S^T[k,q] = K·Q^T in bf16 on
the TensorEngine, injects the boolean mask additively via a second matmul
(mask_block^T @ (-30000·I) accumulated into the same PSUM — transposes the
mask for free), applies exp with the 1/sqrt(128) scale fused into the
ScalarEngine activation (softmax max-shift skipped: scores ~ N(0,1)),
then AV with a ones-column appended to V so the softmax denominator falls
out of the same matmul; normalization is a per-partition reciprocal+scale.
"""

import numpy as np
import ml_dtypes

B, S, D = 16, 2048, 128
N_CORES = 8
BPC = B // N_CORES  # batches per core
P = 128             # partitions
KB = S // P         # 16 k-blocks
QW = 512            # q-tile width (one PSUM bank of f32)
NQB = S // QW       # 4 q-tiles per batch
NQI = QW // P       # 4 q-subblocks per q-tile
NEG = -30000.0

_NC = None
LAST_RESULT = None  # BassKernelResults of the most recent run (for profiling)


def _build_nc(bpc=BPC, s=S):
    import concourse.bacc as bacc
    import concourse.tile as tile
    from concourse import mybir

    BPC_, S_ = bpc, s
    KB = S_ // P
    NQB = S_ // QW

    scale = 1.0 / float(np.sqrt(D))
    bf16 = mybir.dt.bfloat16
    f32 = mybir.dt.float32

    nc = bacc.Bacc()
    qT = nc.declare_dram_parameter("qT", [BPC_, P, S_], bf16, isOutput=False)
    kT = nc.declare_dram_parameter("kT", [BPC_, P, S_], bf16, isOutput=False)
    va = nc.declare_dram_parameter("va", [BPC_, S_, D + 1], bf16, isOutput=False)
    mk = nc.declare_dram_parameter("mask", [BPC_, S_, S_], bf16, isOutput=False)
    negI = nc.declare_dram_parameter("negI", [P, P], bf16, isOutput=False)
    out = nc.declare_dram_parameter("out", [BPC_, S_, D], f32, isOutput=True)

    with tile.TileContext(nc) as tc:
        with (
            tc.tile_pool(name="const", bufs=1) as constp,
            tc.tile_pool(name="qk", bufs=2) as qkp,
            tc.tile_pool(name="vp", bufs=2) as vp,
            tc.tile_pool(name="mp", bufs=2) as mp,
            tc.tile_pool(name="attn", bufs=4) as attnp,
            tc.tile_pool(name="outp", bufs=4) as outp,
            tc.tile_pool(name="rp", bufs=4) as rp,
            tc.tile_pool(name="spsum", bufs=4, space="PSUM") as spsum,
            tc.tile_pool(name="avpsum", bufs=4, space="PSUM") as avpsum,
        ):
            negI_s = constp.tile([P, P], bf16)
            nc.sync.dma_start(out=negI_s[:], in_=negI[:, :])

            for b in range(BPC_):
                qt_s = qkp.tile([P, S_], bf16, tag="qt")
                kt_s = qkp.tile([P, S_], bf16, tag="kt")
                nc.sync.dma_start(out=qt_s[:], in_=qT[b, :, :])
                nc.sync.dma_start(out=kt_s[:], in_=kT[b, :, :])
                va_s = vp.tile([P, KB, D + 1], bf16)
                for kb in range(KB):
                    nc.sync.dma_start(
                        out=va_s[:, kb, :], in_=va[b, kb * P : (kb + 1) * P, :]
                    )
                for qb in range(NQB):
                    mk_s = mp.tile([P, NQI, S_], bf16)
                    for qi in range(NQI):
                        r0 = qb * QW + qi * P
                        nc.sync.dma_start(
                            out=mk_s[:, qi, :], in_=mk[b, r0 : r0 + P, :]
                        )
                    av_ps = [
                        avpsum.tile([P, D + 1], f32, name="av_ps", tag="av")
                        for _ in range(NQI)
                    ]
                    attn_tiles = [None] * KB
                    # software-pipelined: AV matmuls for iteration kb-2 are
                    # emitted after the score matmuls of iteration kb so the
                    # PE never stalls waiting on the exp of the same tile
                    for kb in range(KB + 2):
                        if kb < KB:
                            s_ps = spsum.tile([P, QW], f32)
                            # start=True zeroes the whole 2KB PSUM bank, so the
                            # full-width QK matmul must come first; the per-slice
                            # mask matmuls then accumulate into it
                            nc.tensor.matmul(
                                s_ps[:],
                                lhsT=kt_s[:, kb * P : (kb + 1) * P],
                                rhs=qt_s[:, qb * QW : (qb + 1) * QW],
                                start=True,
                                stop=False,
                            )
                            for qi in range(NQI):
                                nc.tensor.matmul(
                                    s_ps[:, qi * P : (qi + 1) * P],
                                    lhsT=mk_s[:, qi, kb * P : (kb + 1) * P],
                                    rhs=negI_s[:],
                                    start=False,
                                    stop=(qi == NQI - 1),
                                )
                            attn_s = attnp.tile([P, QW], bf16)
                            nc.scalar.activation(
                                attn_s[:],
                                s_ps[:],
                                mybir.ActivationFunctionType.Exp,
                                scale=scale,
                            )
                            attn_tiles[kb] = attn_s
                        if kb >= 2:
                            kprev = kb - 2
                            ats = attn_tiles[kprev]
                            for qi in range(NQI):
                                nc.tensor.matmul(
                                    av_ps[qi][:],
                                    lhsT=ats[:, qi * P : (qi + 1) * P],
                                    rhs=va_s[:, kprev, :],
                                    start=(kprev == 0),
                                    stop=(kprev == KB - 1),
                                )
                    for qi in range(NQI):
                        recip = rp.tile([P, 1], f32)
                        nc.vector.reciprocal(recip[:], av_ps[qi][:, D : D + 1])
                        o_s = outp.tile([P, D], f32)
                        nc.vector.tensor_scalar_mul(
                            o_s[:], av_ps[qi][:, 0:D], recip[:]
                        )
                        r0 = qb * QW + qi * P
                        nc.sync.dma_start(
                            out=out[b, r0 : r0 + P, :], in_=o_s[:]
                        )
    nc.compile()
    return nc


def kernel(q, k, v, mask, _trace=False, _trace_kwargs=None):
    global _NC, LAST_RESULT
    from concourse.bass_utils import run_bass_kernel_spmd

    if _NC is None:
        _NC = _build_nc()

    bf = ml_dtypes.bfloat16
    ones = np.ones((B, S, 1), dtype=np.float32)
    va_full = np.concatenate([np.asarray(v, np.float32), ones], axis=2).astype(bf)
    qT_full = np.ascontiguousarray(
        np.asarray(q, np.float32).transpose(0, 2, 1)
    ).astype(bf)
    kT_full = np.ascontiguousarray(
        np.asarray(k, np.float32).transpose(0, 2, 1)
    ).astype(bf)
    mk_full = np.asarray(mask).astype(bf)
    negI = (NEG * np.eye(P, dtype=np.float32)).astype(bf)

    in_maps = []
    for c in range(N_CORES):
        lo, hi = c * BPC, (c + 1) * BPC
        in_maps.append(
            {
                "qT": qT_full[lo:hi],
                "kT": kT_full[lo:hi],
                "va": va_full[lo:hi],
                "mask": mk_full[lo:hi],
                "negI": negI,
            }
        )

    kw = {}
    if _trace:
        kw["trace"] = True
        if _trace_kwargs:
            kw.update(_trace_kwargs)
    LAST_RESULT = run_bass_kernel_spmd(_NC, in_maps, list(range(N_CORES)), **kw)
    res = LAST_RESULT.results
    return np.concatenate([np.asarray(res[c]["out"]) for c in range(N_CORES)], axis=0)


# revision 11
# speedup vs baseline: 460.4389x; 460.4389x over previous
"""Masked attention on 8 TRN2 NeuronCores — pure data-parallel over batch.

Full inputs:  q,k,v (16,2048,128) f32, mask (16,2048,2048) bool.
Output:       (16,2048,128) f32.

Per core (2 batches): computes transposed scores S^T[k,q] = K·Q^T in bf16 on
the TensorEngine, injects the boolean mask additively via a second matmul
(mask_block^T @ (-30000·I) accumulated into the same PSUM — transposes the
mask for free), applies exp with the 1/sqrt(128) scale fused into the
ScalarEngine activation (softmax max-shift skipped: scores ~ N(0,1)),
then AV with a ones-column appended to V so the softmax denominator falls
out of the same matmul; normalization is a per-partition reciprocal+scale.
"""

import numpy as np
import ml_dtypes

B, S, D = 16, 2048, 128
N_CORES = 8
BPC = B // N_CORES  # batches per core
P = 128             # partitions
KB = S // P         # 16 k-blocks
QW = 512            # q-tile width (one PSUM bank of f32)
NQB = S // QW       # 4 q-tiles per batch
NQI = QW // P       # 4 q-subblocks per q-tile
NEG = -30000.0

_NC = None
LAST_RESULT = None  # BassKernelResults of the most recent run (for profiling)


def _build_nc(bpc=BPC, s=S, repeat=1):
    import concourse.bacc as bacc
    import concourse.tile as tile
    from concourse import mybir

    BPC_, S_ = bpc, s
    KB = S_ // P
    NQB = S_ // QW

    scale = 1.0 / float(np.sqrt(D))
    bf16 = mybir.dt.bfloat16
    f32 = mybir.dt.float32

    nc = bacc.Bacc()
    qT = nc.declare_dram_parameter("qT", [BPC_, P, S_], bf16, isOutput=False)
    kT = nc.declare_dram_parameter("kT", [BPC_, P, S_], bf16, isOutput=False)
    va = nc.declare_dram_parameter("va", [BPC_, S_, D + 1], bf16, isOutput=False)
    mk = nc.declare_dram_parameter("mask", [BPC_, S_, S_], bf16, isOutput=False)
    negI = nc.declare_dram_parameter("negI", [P, P], bf16, isOutput=False)
    out = nc.declare_dram_parameter("out", [BPC_, S_, D], f32, isOutput=True)

    with tile.TileContext(nc) as tc:
        with (
            tc.tile_pool(name="const", bufs=1) as constp,
            tc.tile_pool(name="qk", bufs=2) as qkp,
            tc.tile_pool(name="vp", bufs=2) as vp,
            tc.tile_pool(name="mp", bufs=2) as mp,
            tc.tile_pool(name="attn", bufs=4) as attnp,
            tc.tile_pool(name="outp", bufs=4) as outp,
            tc.tile_pool(name="rp", bufs=4) as rp,
            tc.tile_pool(name="spsum", bufs=4, space="PSUM") as spsum,
            tc.tile_pool(name="avpsum", bufs=4, space="PSUM") as avpsum,
        ):
            negI_s = constp.tile([P, P], bf16)
            nc.sync.dma_start(out=negI_s[:], in_=negI[:, :])

            for _rep in range(repeat):
              for b in range(BPC_):
                qt_s = qkp.tile([P, S_], bf16, tag="qt")
                kt_s = qkp.tile([P, S_], bf16, tag="kt")
                nc.sync.dma_start(out=qt_s[:], in_=qT[b, :, :])
                nc.sync.dma_start(out=kt_s[:], in_=kT[b, :, :])
                va_s = vp.tile([P, KB, D + 1], bf16)
                for kb in range(KB):
                    nc.sync.dma_start(
                        out=va_s[:, kb, :], in_=va[b, kb * P : (kb + 1) * P, :]
                    )
                for qb in range(NQB):
                    mk_s = mp.tile([P, NQI, S_], bf16)
                    for qi in range(NQI):
                        r0 = qb * QW + qi * P
                        nc.sync.dma_start(
                            out=mk_s[:, qi, :], in_=mk[b, r0 : r0 + P, :]
                        )
                    av_ps = [
                        avpsum.tile([P, D + 1], f32, name="av_ps", tag="av")
                        for _ in range(NQI)
                    ]
                    attn_tiles = [None] * KB
                    # software-pipelined: AV matmuls for iteration kb-2 are
                    # emitted after the score matmuls of iteration kb so the
                    # PE never stalls waiting on the exp of the same tile
                    for kb in range(KB + 2):
                        if kb < KB:
                            s_ps = spsum.tile([P, QW], f32)
                            # start=True zeroes the whole 2KB PSUM bank, so the
                            # full-width QK matmul must come first; the per-slice
                            # mask matmuls then accumulate into it
                            nc.tensor.matmul(
                                s_ps[:],
                                lhsT=kt_s[:, kb * P : (kb + 1) * P],
                                rhs=qt_s[:, qb * QW : (qb + 1) * QW],
                                start=True,
                                stop=False,
                            )
                            for qi in range(NQI):
                                nc.tensor.matmul(
                                    s_ps[:, qi * P : (qi + 1) * P],
                                    lhsT=mk_s[:, qi, kb * P : (kb + 1) * P],
                                    rhs=negI_s[:],
                                    start=False,
                                    stop=(qi == NQI - 1),
                                )
                            attn_s = attnp.tile([P, QW], bf16)
                            nc.scalar.activation(
                                attn_s[:],
                                s_ps[:],
                                mybir.ActivationFunctionType.Exp,
                                scale=scale,
                            )
                            attn_tiles[kb] = attn_s
                        if kb >= 2:
                            kprev = kb - 2
                            ats = attn_tiles[kprev]
                            for qi in range(NQI):
                                nc.tensor.matmul(
                                    av_ps[qi][:],
                                    lhsT=ats[:, qi * P : (qi + 1) * P],
                                    rhs=va_s[:, kprev, :],
                                    start=(kprev == 0),
                                    stop=(kprev == KB - 1),
                                )
                    for qi in range(NQI):
                        recip = rp.tile([P, 1], f32)
                        nc.vector.reciprocal(recip[:], av_ps[qi][:, D : D + 1])
                        o_s = outp.tile([P, D], f32)
                        nc.vector.tensor_scalar_mul(
                            o_s[:], av_ps[qi][:, 0:D], recip[:]
                        )
                        r0 = qb * QW + qi * P
                        nc.sync.dma_start(
                            out=out[b, r0 : r0 + P, :], in_=o_s[:]
                        )
    nc.compile()
    return nc


def kernel(q, k, v, mask, _trace=False, _trace_kwargs=None):
    global _NC, LAST_RESULT
    from concourse.bass_utils import run_bass_kernel_spmd

    if _NC is None:
        _NC = _build_nc()

    bf = ml_dtypes.bfloat16
    ones = np.ones((B, S, 1), dtype=np.float32)
    va_full = np.concatenate([np.asarray(v, np.float32), ones], axis=2).astype(bf)
    qT_full = np.ascontiguousarray(
        np.asarray(q, np.float32).transpose(0, 2, 1)
    ).astype(bf)
    kT_full = np.ascontiguousarray(
        np.asarray(k, np.float32).transpose(0, 2, 1)
    ).astype(bf)
    mk_full = np.asarray(mask).astype(bf)
    negI = (NEG * np.eye(P, dtype=np.float32)).astype(bf)

    in_maps = []
    for c in range(N_CORES):
        lo, hi = c * BPC, (c + 1) * BPC
        in_maps.append(
            {
                "qT": qT_full[lo:hi],
                "kT": kT_full[lo:hi],
                "va": va_full[lo:hi],
                "mask": mk_full[lo:hi],
                "negI": negI,
            }
        )

    kw = {}
    if _trace:
        kw["trace"] = True
        if _trace_kwargs:
            kw.update(_trace_kwargs)
    LAST_RESULT = run_bass_kernel_spmd(_NC, in_maps, list(range(N_CORES)), **kw)
    res = LAST_RESULT.results
    return np.concatenate([np.asarray(res[c]["out"]) for c in range(N_CORES)], axis=0)
